# revision 22
# baseline (speedup 1.0000x reference)
"""AttentionBlock (GroupNorm + 8-head self-attention + proj + residual) on 8 trn2 cores.

Sharding: data-parallel over batch B=16 -> 2 samples per core. No collectives.

Per-sample dataflow (C=512 channels, L=1024 pixels, 8 heads x 64 dims):
  - x (C, L) lives as 4 SBUF tiles (128, 1024), channels on partitions.
  - GroupNorm: per-channel mean/var via bn_stats over L; 16-channel group
    aggregation + broadcast-back via tiny mask matmuls on the PE.
  - QKV: q^T,k^T computed as (channels, L) tiles; v computed directly in
    (L, channels) orientation (lhsT = h) so AV needs no transposes.
  - Attention per head pair: S^T = k^T.T @ q^T chunks (row-packed pairs share
    the PE array, K=64 each); exp via ScalarE with scale=1/8 fused, reading
    PSUM (128,2048) in one instruction; AV uses v' = [v | ones] (M=65) so the
    softmax denominator rides along as PSUM row 64; divide via reciprocal +
    DMA partition-broadcast + DVE multiply.
  - proj + bias + residual, write out.

All matmuls run in float32r (full-rate PE mode, ~1e-4 rel err); data stays
fp32 in SBUF (bitcast at the matmul call sites).
"""

import numpy as np

import concourse.bass as bass
import concourse.mybir as mybir
import concourse.tile as tile
from concourse import bacc
from concourse.bass_utils import run_bass_kernel_spmd
from concourse.masks import make_identity

F32 = mybir.dt.float32
F32R = mybir.dt.float32r
AF = mybir.ActivationFunctionType
OP = mybir.AluOpType

B, C, H, W = 16, 512, 32, 32
L = H * W
NH, HD = 8, 64
NG, GS = 32, 16
EPS = 1e-5
N_CORES = 8
BPC = B // N_CORES  # samples per core
P = 128
CK = C // P   # 4 channel chunks
LK = L // P   # 8 pixel chunks
SCALE = HD ** -0.5

_NC_CACHE = {}


def _emit(nc, tc, pools, x_d, out_d, nw_d, nb_d, qw_d, qb_d, pw_d, pb_d):
    const, stage, xp, hp_, qkp, vp, ep, attp, op_, sm, csp, ps = pools

    # ---------------- constants ----------------
    ident = const.tile([P, P], F32, tag="ident")
    make_identity(nc, ident)

    # gmask[kc][c, g] = 1/16 if global_channel // 16 == g else 0   (128, 32)
    gmask = []
    for kc in range(CK):
        gm = const.tile([P, NG], F32, tag=f"gmask{kc}")
        nc.gpsimd.memset(gm, 1.0 / GS)  # averages the 16 per-channel stats of a group
        nc.gpsimd.affine_select(
            out=gm, in_=gm, compare_op=OP.is_ge, fill=0.0,
            base=P * kc, channel_multiplier=1, pattern=[[-GS, NG]])
        nc.gpsimd.affine_select(
            out=gm, in_=gm, compare_op=OP.is_ge, fill=0.0,
            base=(GS - 1) - P * kc, channel_multiplier=-1, pattern=[[GS, NG]])
        gmask.append(gm)

    # selmask[h, c] = 1 if c // 64 == h  (8, 512): head-of-channel selector for
    # broadcasting per-head softmax denominators to channel rows via the PE
    selmask = const.tile([NH, C], F32, tag="selmask")
    nc.gpsimd.memset(selmask, 1.0)
    nc.gpsimd.affine_select(
        out=selmask, in_=selmask, compare_op=OP.is_ge, fill=0.0,
        base=0, channel_multiplier=-HD, pattern=[[1, C]])
    nc.gpsimd.affine_select(
        out=selmask, in_=selmask, compare_op=OP.is_ge, fill=0.0,
        base=HD - 1, channel_multiplier=HD, pattern=[[-1, C]])

    # bmask[g, c] = 1 if c // 16 == g  (32, 512); sliced per chunk as lhsT
    bmask = const.tile([NG, C], F32, tag="bmask")
    nc.gpsimd.memset(bmask, 1.0)
    nc.gpsimd.affine_select(
        out=bmask, in_=bmask, compare_op=OP.is_ge, fill=0.0,
        base=0, channel_multiplier=-GS, pattern=[[1, C]])
    nc.gpsimd.affine_select(
        out=bmask, in_=bmask, compare_op=OP.is_ge, fill=0.0,
        base=GS - 1, channel_multiplier=GS, pattern=[[-1, C]])

    # per-channel params as (128, 1) column tiles
    nw_r = nw_d.ap().rearrange("(kc p) -> kc p", p=P)
    nb_r = nb_d.ap().rearrange("(kc p) -> kc p", p=P)
    pb_r = pb_d.ap().rearrange("(kc p) -> kc p", p=P)
    qb_r = qb_d.ap().rearrange("(oc p) -> oc p", p=P)
    nw, nb, pb, qb = [], [], [], []
    for kc in range(CK):
        t = const.tile([P, 1], F32, tag=f"nw{kc}")
        nc.sync.dma_start(t, nw_r[kc][:, None])
        nw.append(t)
        t = const.tile([P, 1], F32, tag=f"nb{kc}")
        nc.sync.dma_start(t, nb_r[kc][:, None])
        nb.append(t)
        t = const.tile([P, 1], F32, tag=f"pb{kc}")
        nc.sync.dma_start(t, pb_r[kc][:, None])
        pb.append(t)
    for oc in range(8):  # q,k bias chunks
        t = const.tile([P, 1], F32, tag=f"qb{oc}")
        nc.sync.dma_start(t, qb_r[oc][:, None])
        qb.append(t)
    eps_t = const.tile([NG, 1], F32, tag="eps_t")
    nc.vector.memset(eps_t, EPS)
    ones_col = const.tile([P, NH], F32, tag="ones_col")
    nc.vector.memset(ones_col, 1.0)
    # v bias broadcast across partitions (it indexes the free dim of v tiles)
    vb = const.tile([P, 512], F32, tag="vb")
    nc.gpsimd.dma_start(vb[:, None, :], qb_d.ap()[1024:1536][None, :].partition_broadcast(P))

    # ---------------- weight transposes (W -> W^T on the PE) ----------------
    qw_r = qw_d.ap().rearrange("(oc p) c -> oc p c", p=P)   # (12, 128, 512)
    pw_r = pw_d.ap().rearrange("(oc p) c -> oc p c", p=P)   # (4, 128, 512)
    wT = [const.tile([P, 3 * C], F32R, tag=f"wT{kc}", name=f"wT{kc}") for kc in range(CK)]
    pT = [const.tile([P, C], F32R, tag=f"pT{kc}", name=f"pT{kc}") for kc in range(CK)]
    for oc in range(12):
        ws = stage.tile([P, C], F32, tag="wstage")
        nc.sync.dma_start(ws, qw_r[oc])
        pt = ps.tile([P, 2048], F32, tag="ps")
        for kc in range(CK):
            nc.tensor.transpose(pt[:, kc * P:(kc + 1) * P], ws[:, kc * P:(kc + 1) * P], ident)
        for kc in range(CK):
            nc.any.tensor_copy(out=wT[kc][:, oc * P:(oc + 1) * P], in_=pt[:, kc * P:(kc + 1) * P])
    for oc in range(CK):
        ws = stage.tile([P, C], F32, tag="wstage")
        nc.sync.dma_start(ws, pw_r[oc])
        pt = ps.tile([P, 2048], F32, tag="ps")
        for kc in range(CK):
            nc.tensor.transpose(pt[:, kc * P:(kc + 1) * P], ws[:, kc * P:(kc + 1) * P], ident)
        for kc in range(CK):
            nc.any.tensor_copy(out=pT[kc][:, oc * P:(oc + 1) * P], in_=pt[:, kc * P:(kc + 1) * P])

    x_r = x_d.ap().rearrange("b (kc p) h w -> b kc p (h w)", p=P)
    o_r = out_d.ap().rearrange("b (kc p) h w -> b kc p (h w)", p=P)

    for s in range(BPC):
        # ---------------- load x + GroupNorm ----------------
        x_sb = []
        stat2s = []
        for kc in range(CK):
            xt = xp.tile([P, L], F32, tag=f"x{kc}")
            nc.sync.dma_start(xt, x_r[s, kc])
            x_sb.append(xt)
            st = sm.tile([P, 2, 6], F32, tag="bst")
            nc.vector.bn_stats(out=st[:, 0, :], in_=xt[:, 0:512])
            nc.vector.bn_stats(out=st[:, 1, :], in_=xt[:, 512:1024])
            mv = sm.tile([P, 2], F32, tag="mv")
            nc.vector.bn_aggr(out=mv, in_=st)
            # stat2 = [sum-ish mean, E[x^2]] per channel (scaled by gmask later)
            st2 = sm.tile([P, 2], F32, tag="st2")
            nc.vector.tensor_copy(out=st2[:, 0:1], in_=mv[:, 0:1])
            nc.vector.tensor_tensor(st2[:, 1:2], mv[:, 0:1], mv[:, 0:1], OP.mult)
            nc.vector.tensor_tensor(st2[:, 1:2], st2[:, 1:2], mv[:, 1:2], OP.add)
            stat2s.append(st2)
        gps = ps.tile([P, 2048], F32, tag="ps")
        for kc in range(CK):
            nc.tensor.matmul(gps[0:NG, 0:2], gmask[kc], stat2s[kc],
                             start=(kc == 0), stop=(kc == CK - 1))
        # group stats -> [gmean, rstd] on 32 partitions
        gst = sm.tile([NG, 2], F32, tag="gst")
        gsb = sm.tile([NG, 2], F32, tag="gsb")
        gtmp = sm.tile([NG, 1], F32, tag="gtmp")
        nc.vector.tensor_copy(out=gsb, in_=gps[0:NG, 0:2])
        nc.vector.tensor_tensor(gtmp, gsb[:, 0:1], gsb[:, 0:1], OP.mult)
        nc.vector.tensor_tensor(gtmp, gsb[:, 1:2], gtmp, OP.subtract)  # var
        nc.scalar.activation(gtmp, gtmp, AF.Ln, bias=eps_t)
        nc.scalar.activation(gst[:, 1:2], gtmp, AF.Exp, scale=-0.5)       # rstd
        nc.vector.tensor_copy(out=gst[:, 0:1], in_=gsb[:, 0:1])           # gmean
        # broadcast to channels + fold norm_w/norm_b -> per-channel A, B
        chps = ps.tile([P, 2048], F32, tag="ps")
        for kc in range(CK):
            nc.tensor.matmul(chps[:, kc * 2: kc * 2 + 2],
                             bmask[:, kc * P:(kc + 1) * P], gst,
                             start=True, stop=True)
        h_sb = []
        for kc in range(CK):
            Acol = sm.tile([P, 1], F32, tag="Acol")
            Bcol = sm.tile([P, 1], F32, tag="Bcol")
            nc.vector.tensor_tensor(Acol, chps[:, kc * 2 + 1: kc * 2 + 2], nw[kc], OP.mult)
            nc.vector.tensor_tensor(Bcol, chps[:, kc * 2: kc * 2 + 1], Acol, OP.mult)
            nc.vector.tensor_tensor(Bcol, nb[kc], Bcol, OP.subtract)
            ht = hp_.tile([P, L], F32R, tag=f"h{kc}")
            nc.vector.tensor_scalar(ht, x_sb[kc], Acol, Bcol, op0=OP.mult, op1=OP.add)
            h_sb.append(ht)

        # ---------------- QKV ----------------
        # q^T, k^T: (128 chan, 1024 pix) tiles, oc 0..3 = q heads, 4..7 = k heads
        qkT = []
        for oc in range(8):
            dst = qkp.tile([P, L], F32R, tag=f"qk{oc}")
            qkT.append(dst)
            for li in range(2):
                if li == 0:
                    pt = ps.tile([P, 2048], F32, tag="ps", name="qkv_ps")
                for kc in range(CK):
                    nc.tensor.matmul(pt[:, li * 512:(li + 1) * 512],
                                     wT[kc][:, oc * P:(oc + 1) * P],
                                     h_sb[kc][:, li * 512:(li + 1) * 512],
                                     start=(kc == 0), stop=(kc == CK - 1))
            for li in range(2):
                nc.vector.tensor_scalar(dst[:, li * 512:(li + 1) * 512],
                                        pt[:, li * 512:(li + 1) * 512],
                                        qb[oc], None, op0=OP.add)
        # v in (L, channels) orientation, with ones column -> (128, 8, 65) per l-chunk
        v_sb = []
        for lc2 in range(LK // 2):
            pt = ps.tile([P, 2048], F32, tag="ps")
            for half in range(2):
                lc = lc2 * 2 + half
                for kc in range(CK):
                    nc.tensor.matmul(pt[:, half * 1024: half * 1024 + 512],
                                     h_sb[kc][:, lc * P:(lc + 1) * P],
                                     wT[kc][:, 1024:1536],
                                     start=(kc == 0), stop=(kc == CK - 1))
            for half in range(2):
                lc = lc2 * 2 + half
                vt = vp.tile([P, NH, HD + 1], F32R, tag=f"v{lc}")
                nc.vector.tensor_copy(out=vt[:, :, HD:HD + 1], in_=ones_col[:, :, None])
                nc.vector.tensor_tensor(
                    vt[:, :, 0:HD],
                    pt[:, half * 1024: half * 1024 + 512].rearrange("p (h d) -> p h d", d=HD),
                    vb.rearrange("p (h d) -> p h d", d=HD),
                    OP.add)
                v_sb.append(vt)

        # ---------------- attention, head pairs ----------------
        att = [attp.tile([P, L], F32R, tag=f"att{kc}", name=f"att{kc}") for kc in range(CK)]
        csum = csp.tile([NH, L], F32, tag="csum")
        for hp in range(NH // 2):
            av = ps.tile([P, 2048], F32, tag="ps")  # 4 regions: (h_local, ic)
            e_ts = []
            for jc in range(LK):
                st_ = ps.tile([P, 2048], F32, tag="ps")
                for h2 in range(2):
                    for ic in range(2):
                        nc.tensor.matmul(
                            st_[:, h2 * 1024 + ic * 512: h2 * 1024 + (ic + 1) * 512],
                            qkT[4 + hp][h2 * HD:(h2 + 1) * HD, jc * P:(jc + 1) * P],
                            qkT[hp][h2 * HD:(h2 + 1) * HD, ic * 512:(ic + 1) * 512],
                            start=True, stop=True)
                e_t = ep.tile([P, 2048], F32R, tag="e")
                nc.scalar.activation(e_t, st_, AF.Exp, scale=SCALE)
                e_ts.append(e_t)
                for h2 in range(2):
                    for ic in range(2):
                        r = h2 * 2 + ic
                        nc.tensor.matmul(
                            av[0:HD + 1, r * 512:(r + 1) * 512],
                            v_sb[jc][:, 2 * hp + h2, :],
                            e_t[:, h2 * 1024 + ic * 512: h2 * 1024 + (ic + 1) * 512],
                            start=(jc == 0), stop=(jc == LK - 1))
            # move unnormalized AV to att rows; stash colsums (PSUM row 64)
            for h2 in range(2):
                h = 2 * hp + h2
                nc.vector.tensor_copy(
                    out=att[hp][h2 * HD:(h2 + 1) * HD, :],
                    in_=av[0:HD, h2 * 1024:(h2 + 1) * 1024])
                cstage = sm.tile([1, L], F32, tag="cstage")
                nc.vector.tensor_copy(
                    out=cstage, in_=av[HD:HD + 1, h2 * 1024:(h2 + 1) * 1024])
                nc.sync.dma_start(csum[h:h + 1, :], cstage)

        # normalize: recip denominators, broadcast head->channel rows on the PE
        rsum = csp.tile([NH, L], F32, tag="rsum")
        nc.vector.reciprocal(rsum, csum)
        for half in range(2):
            rb2 = ps.tile([P, 2048], F32, tag="ps", name="rb2_ps")
            for q in range(2):
                kc = half * 2 + q
                for li in range(2):
                    nc.tensor.matmul(
                        rb2[:, q * 1024 + li * 512: q * 1024 + (li + 1) * 512],
                        selmask[:, kc * P:(kc + 1) * P],
                        rsum[:, li * 512:(li + 1) * 512],
                        start=True, stop=True)
            for q in range(2):
                kc = half * 2 + q
                nc.vector.tensor_tensor(
                    att[kc], att[kc], rb2[:, q * 1024:(q + 1) * 1024], OP.mult)

        # ---------------- proj + bias + residual ----------------
        for half in range(2):
            pt = ps.tile([P, 2048], F32, tag="ps")
            for q in range(4):
                oc, li = (half * 4 + q) // 2, (half * 4 + q) % 2
                for kc in range(CK):
                    nc.tensor.matmul(pt[:, q * 512:(q + 1) * 512],
                                     pT[kc][:, oc * P:(oc + 1) * P],
                                     att[kc][:, li * 512:(li + 1) * 512],
                                     start=(kc == 0), stop=(kc == CK - 1))
            for q in range(4):
                oc, li = (half * 4 + q) // 2, (half * 4 + q) % 2
                ot = op_.tile([P, 512], F32, tag="ot")
                nc.vector.tensor_scalar(ot, pt[:, q * 512:(q + 1) * 512],
                                        pb[oc], None, op0=OP.add)
                nc.vector.tensor_tensor(ot, ot, x_sb[oc][:, li * 512:(li + 1) * 512], OP.add)
                nc.sync.dma_start(o_r[s, oc][:, li * 512:(li + 1) * 512], ot)


def _build():
    if "nc" in _NC_CACHE:
        return _NC_CACHE["nc"]
    nc = bacc.Bacc("TRN2", target_bir_lowering=False, debug=False)
    x_d = nc.dram_tensor("x", (BPC, C, H, W), F32, kind="ExternalInput")
    nw_d = nc.dram_tensor("norm_w", (C,), F32, kind="ExternalInput")
    nb_d = nc.dram_tensor("norm_b", (C,), F32, kind="ExternalInput")
    qw_d = nc.dram_tensor("qkv_w", (3 * C, C), F32, kind="ExternalInput")
    qb_d = nc.dram_tensor("qkv_b", (3 * C,), F32, kind="ExternalInput")
    pw_d = nc.dram_tensor("proj_w", (C, C), F32, kind="ExternalInput")
    pb_d = nc.dram_tensor("proj_b", (C,), F32, kind="ExternalInput")
    out_d = nc.dram_tensor("out", (BPC, C, H, W), F32, kind="ExternalOutput")
    with tile.TileContext(nc) as tc:
        with (
            tc.tile_pool(name="const", bufs=1) as const,
            tc.tile_pool(name="stage", bufs=3) as stage,
            tc.tile_pool(name="xp", bufs=2) as xp,
            tc.tile_pool(name="hp", bufs=1) as hp_,
            tc.tile_pool(name="qkp", bufs=1) as qkp,
            tc.tile_pool(name="vp", bufs=1) as vp,
            tc.tile_pool(name="ep", bufs=2) as ep,
            tc.tile_pool(name="attp", bufs=1) as attp,
            tc.tile_pool(name="op", bufs=2) as op_,
            tc.tile_pool(name="sm", bufs=4) as sm,
            tc.tile_pool(name="csp", bufs=1) as csp,
            tc.tile_pool(name="ps", bufs=2, space="PSUM") as ps,
        ):
            pools = (const, stage, xp, hp_, qkp, vp, ep, attp, op_, sm, csp, ps)
            _emit(nc, tc, pools, x_d, out_d, nw_d, nb_d, qw_d, qb_d, pw_d, pb_d)
    nc.compile()
    _NC_CACHE["nc"] = nc
    return nc


def kernel(x, norm_w, norm_b, qkv_w, qkv_b, proj_w, proj_b):
    x = np.ascontiguousarray(x, dtype=np.float32)
    args = {
        "norm_w": np.ascontiguousarray(norm_w, np.float32),
        "norm_b": np.ascontiguousarray(norm_b, np.float32),
        "qkv_w": np.ascontiguousarray(qkv_w, np.float32),
        "qkv_b": np.ascontiguousarray(qkv_b, np.float32),
        "proj_w": np.ascontiguousarray(proj_w, np.float32),
        "proj_b": np.ascontiguousarray(proj_b, np.float32),
    }
    nc = _build()
    in_maps = [dict(args, x=x[i * BPC:(i + 1) * BPC]) for i in range(N_CORES)]
    res = run_bass_kernel_spmd(nc, in_maps, core_ids=list(range(N_CORES)))
    return np.concatenate([r["out"] for r in res.results], axis=0)


# revision 24
# speedup vs baseline: 1.3054x; 1.3054x over previous
"""AttentionBlock (GroupNorm + 8-head self-attention + proj + residual) on 8 trn2 cores.

Sharding: data-parallel over batch B=16 -> 2 samples per core. No collectives.

Per-sample dataflow (C=512 channels, L=1024 pixels, 8 heads x 64 dims):
  - x (C, L) lives as 4 SBUF tiles (128, 1024), channels on partitions.
  - GroupNorm: per-channel mean/var via bn_stats over L; 16-channel group
    aggregation + broadcast-back via tiny mask matmuls on the PE.
  - QKV: q^T,k^T computed as (channels, L) tiles; v computed directly in
    (L, channels) orientation (lhsT = h) so AV needs no transposes.
  - Attention per head pair, split by i-halves so PSUM double-buffers:
    S^T = k^T.T @ q^T chunks (row-packed head pairs share the PE array, K=64
    each); exp on ScalarE with the 1/8 scale fused, PSUM (128,1024) in one
    instruction; AV uses v' = [v | ones] (M=65) so the softmax denominator
    rides along as PSUM row 64. Denominators for all 8 heads collect into an
    (8, L) tile; one reciprocal + a selector matmul broadcasts them back to
    channel rows for a single normalization multiply per chunk.
  - proj + bias + residual, write out.
  - Sample s+1's groupnorm/QKV are emitted between sample s's attention pairs
    so the PE fills the gaps while ScalarE works through the exps.

Big matmuls run in float32r (full-rate PE mode, ~1e-4 rel err); producers of
their operands write float32r-rounded outputs as walrus requires. Tiny mask
matmuls stay plain fp32.
"""

import numpy as np

import concourse.bass as bass
import concourse.mybir as mybir
import concourse.tile as tile
from concourse import bacc
from concourse.bass_utils import run_bass_kernel_spmd
from concourse.masks import make_identity

F32 = mybir.dt.float32
F32R = mybir.dt.float32r
AF = mybir.ActivationFunctionType
OP = mybir.AluOpType

B, C, H, W = 16, 512, 32, 32
L = H * W
NH, HD = 8, 64
NG, GS = 32, 16
EPS = 1e-5
N_CORES = 8
BPC = B // N_CORES  # samples per core
P = 128
CK = C // P   # 4 channel chunks
LK = L // P   # 8 pixel chunks
SCALE = HD ** -0.5

_NC_CACHE = {}


class Ctx:
    pass


def _consts(nc, const, nw_d, nb_d, qw_d, qb_d, pw_d, pb_d, stage, ps2):
    c = Ctx()
    c.ident = const.tile([P, P], F32, tag="ident")
    make_identity(nc, c.ident)

    # gmask[kc][ch, g] = 1/16 iff global_channel // 16 == g   (128, 32)
    c.gmask = []
    for kc in range(CK):
        gm = const.tile([P, NG], F32, tag=f"gmask{kc}", name=f"gmask{kc}")
        nc.gpsimd.memset(gm, 1.0 / GS)
        nc.gpsimd.affine_select(
            out=gm, in_=gm, compare_op=OP.is_ge, fill=0.0,
            base=P * kc, channel_multiplier=1, pattern=[[-GS, NG]])
        nc.gpsimd.affine_select(
            out=gm, in_=gm, compare_op=OP.is_ge, fill=0.0,
            base=(GS - 1) - P * kc, channel_multiplier=-1, pattern=[[GS, NG]])
        c.gmask.append(gm)

    # selmask[h, ch] = 1 iff ch // 64 == h  (8, 512)
    c.selmask = const.tile([NH, C], F32, tag="selmask")
    nc.gpsimd.memset(c.selmask, 1.0)
    nc.gpsimd.affine_select(
        out=c.selmask, in_=c.selmask, compare_op=OP.is_ge, fill=0.0,
        base=0, channel_multiplier=-HD, pattern=[[1, C]])
    nc.gpsimd.affine_select(
        out=c.selmask, in_=c.selmask, compare_op=OP.is_ge, fill=0.0,
        base=HD - 1, channel_multiplier=HD, pattern=[[-1, C]])

    # bmask[g, ch] = 1 iff ch // 16 == g  (32, 512)
    c.bmask = const.tile([NG, C], F32, tag="bmask")
    nc.gpsimd.memset(c.bmask, 1.0)
    nc.gpsimd.affine_select(
        out=c.bmask, in_=c.bmask, compare_op=OP.is_ge, fill=0.0,
        base=0, channel_multiplier=-GS, pattern=[[1, C]])
    nc.gpsimd.affine_select(
        out=c.bmask, in_=c.bmask, compare_op=OP.is_ge, fill=0.0,
        base=GS - 1, channel_multiplier=GS, pattern=[[-1, C]])

    nw_r = nw_d.ap().rearrange("(kc p) -> kc p", p=P)
    nb_r = nb_d.ap().rearrange("(kc p) -> kc p", p=P)
    pb_r = pb_d.ap().rearrange("(kc p) -> kc p", p=P)
    qb_r = qb_d.ap().rearrange("(oc p) -> oc p", p=P)
    c.nw, c.nb, c.pb, c.qb = [], [], [], []
    for kc in range(CK):
        t = const.tile([P, 1], F32, tag=f"nw{kc}", name=f"nw{kc}")
        nc.sync.dma_start(t, nw_r[kc][:, None])
        c.nw.append(t)
        t = const.tile([P, 1], F32, tag=f"nb{kc}", name=f"nb{kc}")
        nc.sync.dma_start(t, nb_r[kc][:, None])
        c.nb.append(t)
        t = const.tile([P, 1], F32, tag=f"pb{kc}", name=f"pb{kc}")
        nc.sync.dma_start(t, pb_r[kc][:, None])
        c.pb.append(t)
    for oc in range(8):
        t = const.tile([P, 1], F32, tag=f"qb{oc}", name=f"qb{oc}")
        nc.sync.dma_start(t, qb_r[oc][:, None])
        c.qb.append(t)
    c.eps_t = const.tile([NG, 1], F32, tag="eps_t")
    nc.vector.memset(c.eps_t, EPS)
    c.ones_col = const.tile([P, NH], F32, tag="ones_col")
    nc.vector.memset(c.ones_col, 1.0)
    # v bias broadcast across partitions (it indexes the free dim of v tiles)
    c.vb = const.tile([P, 512], F32, tag="vb")
    nc.gpsimd.dma_start(
        c.vb[:, None, :], qb_d.ap()[1024:1536][None, :].partition_broadcast(P))

    # W -> W^T via PE transpose
    qw_r = qw_d.ap().rearrange("(oc p) ch -> oc p ch", p=P)
    pw_r = pw_d.ap().rearrange("(oc p) ch -> oc p ch", p=P)
    c.wT = [const.tile([P, 3 * C], F32R, tag=f"wT{kc}", name=f"wT{kc}")
            for kc in range(CK)]
    c.pT = [const.tile([P, C], F32R, tag=f"pT{kc}", name=f"pT{kc}")
            for kc in range(CK)]
    for oc in range(12):
        ws = stage.tile([P, C], F32, tag="wstage", name="wstage")
        nc.sync.dma_start(ws, qw_r[oc])
        pt = ps2.tile([P, 512], F32, tag="p2", name="tr_ps")
        for kc in range(CK):
            nc.tensor.transpose(pt[:, kc * P:(kc + 1) * P],
                                ws[:, kc * P:(kc + 1) * P], c.ident)
        for kc in range(CK):
            nc.any.tensor_copy(out=c.wT[kc][:, oc * P:(oc + 1) * P],
                               in_=pt[:, kc * P:(kc + 1) * P])
    for oc in range(CK):
        ws = stage.tile([P, C], F32, tag="wstage", name="wstage")
        nc.sync.dma_start(ws, pw_r[oc])
        pt = ps2.tile([P, 512], F32, tag="p2", name="tr_ps")
        for kc in range(CK):
            nc.tensor.transpose(pt[:, kc * P:(kc + 1) * P],
                                ws[:, kc * P:(kc + 1) * P], c.ident)
        for kc in range(CK):
            nc.any.tensor_copy(out=c.pT[kc][:, oc * P:(oc + 1) * P],
                               in_=pt[:, kc * P:(kc + 1) * P])
    return c


def _emit(nc, tc, pools, x_d, out_d, nw_d, nb_d, qw_d, qb_d, pw_d, pb_d):
    const, stage, xp, hp_, qkp, vp, ep, attp, op_, sm, csp, ps, ps2 = pools
    c = _consts(nc, const, nw_d, nb_d, qw_d, qb_d, pw_d, pb_d, stage, ps2)

    x_r = x_d.ap().rearrange("b (kc p) h w -> b kc p (h w)", p=P)
    o_r = out_d.ap().rearrange("b (kc p) h w -> b kc p (h w)", p=P)

    S = [Ctx() for _ in range(BPC)]

    def emit_gn(s):
        st_ = S[s]
        st_.x, stat2s = [], []
        for kc in range(CK):
            xt = xp.tile([P, L], F32, tag=f"x{kc}", name=f"x{kc}_{s}")
            nc.sync.dma_start(xt, x_r[s, kc])
            st_.x.append(xt)
            bst = sm.tile([P, 2, 6], F32, tag="bst", name="bst")
            nc.vector.bn_stats(out=bst[:, 0, :], in_=xt[:, 0:512])
            nc.vector.bn_stats(out=bst[:, 1, :], in_=xt[:, 512:1024])
            mv = sm.tile([P, 2], F32, tag="mv", name="mv")
            nc.vector.bn_aggr(out=mv, in_=bst)
            st2 = sm.tile([P, 2], F32, tag="st2", name="st2")
            nc.vector.tensor_copy(out=st2[:, 0:1], in_=mv[:, 0:1])
            nc.vector.tensor_tensor(st2[:, 1:2], mv[:, 0:1], mv[:, 0:1], OP.mult)
            nc.vector.tensor_tensor(st2[:, 1:2], st2[:, 1:2], mv[:, 1:2], OP.add)
            stat2s.append(st2)
        gps = ps2.tile([P, 512], F32, tag="p2", name="gn_ps")
        for kc in range(CK):
            nc.tensor.matmul(gps[0:NG, 0:2], c.gmask[kc], stat2s[kc],
                             start=(kc == 0), stop=(kc == CK - 1))
        gst = sm.tile([NG, 2], F32, tag="gst", name="gst")
        gsb = sm.tile([NG, 2], F32, tag="gsb", name="gsb")
        gtmp = sm.tile([NG, 1], F32, tag="gtmp", name="gtmp")
        nc.vector.tensor_copy(out=gsb, in_=gps[0:NG, 0:2])
        nc.vector.tensor_tensor(gtmp, gsb[:, 0:1], gsb[:, 0:1], OP.mult)
        nc.vector.tensor_tensor(gtmp, gsb[:, 1:2], gtmp, OP.subtract)  # var
        nc.scalar.activation(gtmp, gtmp, AF.Ln, bias=c.eps_t)
        nc.scalar.activation(gst[:, 1:2], gtmp, AF.Exp, scale=-0.5)    # rstd
        nc.vector.tensor_copy(out=gst[:, 0:1], in_=gsb[:, 0:1])        # gmean
        chps = ps2.tile([P, 512], F32, tag="p2", name="gn_ps2")
        for kc in range(CK):
            nc.tensor.matmul(chps[:, kc * 2: kc * 2 + 2],
                             c.bmask[:, kc * P:(kc + 1) * P], gst,
                             start=True, stop=True)
        st_.h = []
        for kc in range(CK):
            Acol = sm.tile([P, 1], F32, tag="Acol", name="Acol")
            Bcol = sm.tile([P, 1], F32, tag="Bcol", name="Bcol")
            nc.vector.tensor_tensor(Acol, chps[:, kc * 2 + 1: kc * 2 + 2],
                                    c.nw[kc], OP.mult)
            nc.vector.tensor_tensor(Bcol, chps[:, kc * 2: kc * 2 + 1], Acol, OP.mult)
            nc.vector.tensor_tensor(Bcol, c.nb[kc], Bcol, OP.subtract)
            ht = hp_.tile([P, L], F32R, tag=f"h{kc}", name=f"h{kc}_{s}")
            nc.vector.tensor_scalar(ht, st_.x[kc], Acol, Bcol, op0=OP.mult, op1=OP.add)
            st_.h.append(ht)
        st_.qkT = [None] * 8
        st_.v = [None] * LK
        st_.att = [attp.tile([P, L], F32R, tag=f"att{kc}", name=f"att{kc}_{s}")
                   for kc in range(CK)]
        st_.csum = csp.tile([NH, L], F32, tag="csum", name=f"csum{s}")

    def emit_qkv_qk(s, hp):
        st_ = S[s]
        for oc in (hp, 4 + hp):
            dst = qkp.tile([P, L], F32R, tag=f"qk{oc}", name=f"qk{oc}_{s}")
            st_.qkT[oc] = dst
            pt0 = ps2.tile([P, 512], F32, tag="p2", name="qkv_ps0")
            pt1 = ps2.tile([P, 512], F32, tag="p2", name="qkv_ps1")
            pts = [pt0, pt1]
            for kc in range(CK):
                for li in range(2):
                    nc.tensor.matmul(pts[li],
                                     c.wT[kc][:, oc * P:(oc + 1) * P],
                                     st_.h[kc][:, li * 512:(li + 1) * 512],
                                     start=(kc == 0), stop=(kc == CK - 1))
            for li in range(2):
                nc.vector.tensor_scalar(dst[:, li * 512:(li + 1) * 512],
                                        pts[li], c.qb[oc], None, op0=OP.add)

    def emit_v(s):
        st_ = S[s]
        for lc in range(LK):
            pt = ps2.tile([P, 512], F32, tag="p2", name="v_ps")
            for kc in range(CK):
                nc.tensor.matmul(pt,
                                 st_.h[kc][:, lc * P:(lc + 1) * P],
                                 c.wT[kc][:, 1024:1536],
                                 start=(kc == 0), stop=(kc == CK - 1))
            vt = vp.tile([P, NH, HD + 1], F32R, tag=f"v{lc}", name=f"v{lc}_{s}")
            nc.vector.tensor_copy(out=vt[:, :, HD:HD + 1], in_=c.ones_col[:, :, None])
            nc.vector.tensor_tensor(
                vt[:, :, 0:HD],
                pt.rearrange("p (h d) -> p h d", d=HD),
                c.vb.rearrange("p (h d) -> p h d", d=HD),
                OP.add)
            st_.v[lc] = vt

    def emit_pair(s, hp):
        st_ = S[s]
        kT, qT = st_.qkT[4 + hp], st_.qkT[hp]
        for ic in range(2):
            av = ps.tile([P, 1024], F32, tag="s", name=f"av_{hp}_{ic}")
            for jc in range(LK):
                stile = ps.tile([P, 1024], F32, tag="s", name=f"s_{hp}_{ic}_{jc}")
                for h2 in range(2):
                    nc.tensor.matmul(
                        stile[:, h2 * 512:(h2 + 1) * 512],
                        kT[h2 * HD:(h2 + 1) * HD, jc * P:(jc + 1) * P],
                        qT[h2 * HD:(h2 + 1) * HD, ic * 512:(ic + 1) * 512],
                        start=True, stop=True)
                e_t = ep.tile([P, 1024], F32R, tag="e", name="e_t")
                nc.scalar.activation(e_t, stile, AF.Exp, scale=SCALE)
                for h2 in range(2):
                    nc.tensor.matmul(
                        av[0:HD + 1, h2 * 512:(h2 + 1) * 512],
                        st_.v[jc][:, 2 * hp + h2, :],
                        e_t[:, h2 * 512:(h2 + 1) * 512],
                        start=(jc == 0), stop=(jc == LK - 1))
            for h2 in range(2):
                h = 2 * hp + h2
                nc.vector.tensor_copy(
                    out=st_.att[hp][h2 * HD:(h2 + 1) * HD, ic * 512:(ic + 1) * 512],
                    in_=av[0:HD, h2 * 512:(h2 + 1) * 512])
                cstage = sm.tile([1, 512], F32, tag="cstage", name="cstage")
                nc.vector.tensor_copy(
                    out=cstage, in_=av[HD:HD + 1, h2 * 512:(h2 + 1) * 512])
                nc.sync.dma_start(st_.csum[h:h + 1, ic * 512:(ic + 1) * 512], cstage)

    def emit_norm_proj(s):
        st_ = S[s]
        rsum = csp.tile([NH, L], F32, tag="rsum", name=f"rsum{s}")
        nc.vector.reciprocal(rsum, st_.csum)
        for kc in range(CK):
            for li in range(2):
                rb2 = ps2.tile([P, 512], F32, tag="p2", name="rb2_ps")
                nc.tensor.matmul(rb2, c.selmask[:, kc * P:(kc + 1) * P],
                                 rsum[:, li * 512:(li + 1) * 512],
                                 start=True, stop=True)
                nc.vector.tensor_tensor(
                    st_.att[kc][:, li * 512:(li + 1) * 512],
                    st_.att[kc][:, li * 512:(li + 1) * 512], rb2, OP.mult)
        for oc in range(CK):
            pt0 = ps2.tile([P, 512], F32, tag="p2", name="proj_ps0")
            pt1 = ps2.tile([P, 512], F32, tag="p2", name="proj_ps1")
            pts = [pt0, pt1]
            for kc in range(CK):
                for li in range(2):
                    nc.tensor.matmul(pts[li],
                                     c.pT[kc][:, oc * P:(oc + 1) * P],
                                     st_.att[kc][:, li * 512:(li + 1) * 512],
                                     start=(kc == 0), stop=(kc == CK - 1))
            for li in range(2):
                ot = op_.tile([P, 512], F32, tag="ot", name="ot")
                nc.vector.tensor_scalar(ot, pts[li], c.pb[oc], None, op0=OP.add)
                nc.vector.tensor_tensor(ot, ot,
                                        st_.x[oc][:, li * 512:(li + 1) * 512], OP.add)
                nc.sync.dma_start(o_r[s, oc][:, li * 512:(li + 1) * 512], ot)

    # ---- schedule: sample s+1's gn/qkv interleaves sample s's attention ----
    emit_gn(0)
    for hp in range(4):
        emit_qkv_qk(0, hp)
    emit_v(0)
    for s in range(BPC):
        for hp in range(4):
            emit_pair(s, hp)
            if s + 1 < BPC:
                if hp == 0:
                    emit_gn(s + 1)
                emit_qkv_qk(s + 1, hp)
        if s + 1 < BPC:
            emit_v(s + 1)
        emit_norm_proj(s)


def _build():
    if "nc" in _NC_CACHE:
        return _NC_CACHE["nc"]
    nc = bacc.Bacc("TRN2", target_bir_lowering=False, debug=False)
    x_d = nc.dram_tensor("x", (BPC, C, H, W), F32, kind="ExternalInput")
    nw_d = nc.dram_tensor("norm_w", (C,), F32, kind="ExternalInput")
    nb_d = nc.dram_tensor("norm_b", (C,), F32, kind="ExternalInput")
    qw_d = nc.dram_tensor("qkv_w", (3 * C, C), F32, kind="ExternalInput")
    qb_d = nc.dram_tensor("qkv_b", (3 * C,), F32, kind="ExternalInput")
    pw_d = nc.dram_tensor("proj_w", (C, C), F32, kind="ExternalInput")
    pb_d = nc.dram_tensor("proj_b", (C,), F32, kind="ExternalInput")
    out_d = nc.dram_tensor("out", (BPC, C, H, W), F32, kind="ExternalOutput")
    with tile.TileContext(nc) as tc:
        with (
            tc.tile_pool(name="const", bufs=1) as const,
            tc.tile_pool(name="stage", bufs=3) as stage,
            tc.tile_pool(name="xp", bufs=2) as xp,
            tc.tile_pool(name="hp", bufs=1) as hp_,
            tc.tile_pool(name="qkp", bufs=1) as qkp,
            tc.tile_pool(name="vp", bufs=1) as vp,
            tc.tile_pool(name="ep", bufs=3) as ep,
            tc.tile_pool(name="attp", bufs=1) as attp,
            tc.tile_pool(name="op", bufs=2) as op_,
            tc.tile_pool(name="sm", bufs=4) as sm,
            tc.tile_pool(name="csp", bufs=1) as csp,
            tc.tile_pool(name="ps", bufs=3, space="PSUM") as ps,
            tc.tile_pool(name="ps2", bufs=2, space="PSUM") as ps2,
        ):
            pools = (const, stage, xp, hp_, qkp, vp, ep, attp, op_, sm, csp, ps, ps2)
            _emit(nc, tc, pools, x_d, out_d, nw_d, nb_d, qw_d, qb_d, pw_d, pb_d)
    nc.compile()
    _NC_CACHE["nc"] = nc
    return nc


def kernel(x, norm_w, norm_b, qkv_w, qkv_b, proj_w, proj_b):
    x = np.ascontiguousarray(x, dtype=np.float32)
    args = {
        "norm_w": np.ascontiguousarray(norm_w, np.float32),
        "norm_b": np.ascontiguousarray(norm_b, np.float32),
        "qkv_w": np.ascontiguousarray(qkv_w, np.float32),
        "qkv_b": np.ascontiguousarray(qkv_b, np.float32),
        "proj_w": np.ascontiguousarray(proj_w, np.float32),
        "proj_b": np.ascontiguousarray(proj_b, np.float32),
    }
    nc = _build()
    in_maps = [dict(args, x=x[i * BPC:(i + 1) * BPC]) for i in range(N_CORES)]
    res = run_bass_kernel_spmd(nc, in_maps, core_ids=list(range(N_CORES)))
    return np.concatenate([r["out"] for r in res.results], axis=0)
